# revision 56
# baseline (speedup 1.0000x reference)
"""Trainium2 Bass kernel for a 2-layer hetero RGAT (DGL-style), 8 NeuronCores.

Sharding: edges dst-sorted, sharded by contiguous 2560-dst ranges per core
(edge/graph partition parallelism, per-dst segment sums stay core-local; the
only collective is an AllGather of the small layer-1 node features).

Layer 1 (per etype conv):
  table = x_src @ [W | W.al]  -> DRAM [20480, 384] f16 rows [fs(256)|el(8)|pad]
  er    = x_dst_local @ (W.ar)              (per-core local dst blocks)
  SWDGE dma_gather of 768B table rows by u (8-chunk groups of 128 edges)
  per 128-edge chunk:
    er_edge = P01T.T @ er_block             (PE, host-built fp8 0/1 one-hot)
    l = el + er_edge ; w = exp(max(l, .2l)) (DVE + ACT)  == exp(leaky_relu(l))
    G[:, :256] *= w broadcast over d        (DVE, (d,h) col order keeps 2x)
    psum[block] += P01.T @ G[:, :264]       (PE fp8 one-hot lhsT; cols 256:264=w)
  per 128-dst block: acc += psum[:, :256] / max(psum[:, 256:264], eps); the
  h-stage (head-sum + bias + relu + PE transpose into hT) is folded into the
  conv tail per 4-block group so the layer transition stays off the DMA
  critical path.

Layer 2 exploits linearity of the GAT numerator: out2[c,h,:] =
(sum_e w_e h[u_e]) @ W2[:,h]. So the gather only needs raw h rows: one fused
table per node type, rows [h(32 f16) | el2 per src etype (8 each)] at 512B
stride / 256B gather payload (elem_step > elem_size keeps stores above the
sub-512B RMW penalty). Per chunk the rhs is the outer product w x h built at
DVE 2x via a duplicated-pair weight tile ((h, d') order, innermost [1,2] on
all operands); aggregation psum is [c, (h,d')(256) | w(8)] as in layer 1.
Per 4-block mini-tail: divide by the denominator, PE-transpose, and project
with per-head W2 (head-mean folded into w2s = perm(W2)/64).  Softmax
max-subtraction is skipped (exactly equivalent; no overflow at these scales).
h' = relu(sum_h(acc) + 8*mean_h(bias)) = 8*h with the 1/8 folded into the
layer-2 weights.

Perf notes: one-hot p01/p01t host-packed into one contiguous fp8 array (one
full-speed DMA per slice); projection lhs loaded in whole-core 2560-column
chunks; per-block chunk counts (max across cores); table builds batch 4
blocks per PSUM tile; PSUM rings chosen so conv-time tags (pse/psb) never
interleave with transition-time tags (pp/erp), which would serialize phases
through ring order; G/Gw share one 5-slot ring; outputs are produced
transposed [D, DPC] and untransposed on the host.
"""

import sys
import numpy as np
import ml_dtypes

sys.path.insert(0, "/opt/trn_rl_repo")

F16 = np.float16
FP8 = ml_dtypes.float8_e4m3

FIN = 256
H, D = 8, 32
HD = H * D
TW = 384           # l1 table row = 768B (gather elem must be a 256B multiple)
T2W = 128          # l2 table gather payload 256B: [h(32) | el2 (8/etype)...]
T2S = 256          # l2 table row stride (512B so stores avoid the RMW penalty)
WCOL = 272         # packed weight cols: [W(256, (d,h) order) | W.al(8) | W.ar(8)]
ELOFF = HD
NCORE = 8
BLK = 128
NEG = 0.2
DEN_EPS = 1e-30
GSUB = 8           # chunks per dma_gather (SWDGE ring cap 1024 descs)
KST = 10           # projection tiles staged per table-store DMA
SL = 16            # chunks per conv slice


def configure(n=20000, nblk=20):
    """Set problem scale (module globals). Default = full problem."""
    global N, NBLK, DPC, TN, NT
    N = n
    NBLK = nblk
    DPC = NBLK * BLK
    TN = DPC * NCORE
    NT = TN // BLK


configure()

# host col permutation: new col d*H+h <- old col h*D+d
_PERM = np.arange(HD).reshape(H, D).T.reshape(-1)


def _prep_graph(u, v):
    """Sort by dst, shard to cores by 2560-dst ranges.

    Per dst block b the chunk count kb[b] = max over cores of ceil(cnt/128)
    (the program is SPMD so the chunk structure must be shared).
    Returns (kb, per-core dict(u16, pp)).
    """
    order = np.argsort(v, kind="stable")
    us = u[order].astype(np.int64)
    vs = v[order].astype(np.int64)
    core_of = vs // DPC
    per_core = []
    cnts = np.zeros((NCORE, NBLK), np.int64)
    for c in range(NCORE):
        sel = core_of == c
        uc = us[sel]
        vc = vs[sel] - c * DPC
        blk = vc // BLK
        cnts[c] = np.bincount(blk, minlength=NBLK)
        per_core.append((uc, vc))
    kb = np.maximum(1, -(-cnts.max(axis=0) // BLK))  # [NBLK]
    nch = int(kb.sum())
    coff = np.concatenate([[0], np.cumsum(kb)])      # chunk offset per block
    out = []
    for c in range(NCORE):
        uc, vc = per_core[c]
        ebase = np.concatenate([[0], np.cumsum(cnts[c])])
        u_pad = np.zeros(nch * BLK, np.int32)
        eidx = np.full(nch * BLK, -1, np.int64)  # sorted-edge id per slot
        for b in range(NBLK):
            n = int(cnts[c][b])
            dst = int(coff[b]) * BLK
            u_pad[dst:dst + n] = uc[ebase[b]:ebase[b + 1]]
            eidx[dst:dst + n] = np.arange(ebase[b], ebase[b + 1])
        valid = eidx >= 0
        slot = np.arange(nch * BLK)
        ch = slot[valid] // BLK
        row = slot[valid] % BLK
        col = (vc[eidx[valid]] % BLK).astype(np.int64)
        # pp packs p01 ([edge, dst], col block 2*ch) and p01t ([dst, edge],
        # col block 2*ch+1) so one contiguous DMA per slice loads both.
        pp = np.zeros((BLK, nch * 2 * BLK), np.uint8)
        pp[row, ch * 2 * BLK + col] = 0x38             # 1.0 in fp8e4m3
        pp[col, ch * 2 * BLK + BLK + row] = 0x38
        u16 = np.tile(u_pad.astype(np.int16).reshape(-1, 16).T, (8, 1)).copy()
        out.append(dict(
            u16=np.ascontiguousarray(u16),             # [128, nch*8]
            pp=np.ascontiguousarray(pp).view(FP8),     # [128, nch*256]
        ))
    return tuple(int(k) for k in kb), out


def _fold(W, a):
    return np.einsum("ihd,hd->ih", W.reshape(W.shape[0], H, D), a)


def _chunk_meta(kb):
    """[(block, first, last)] per chunk."""
    meta = []
    for b, k in enumerate(kb):
        for j in range(k):
            meta.append((b, j == 0, j == k - 1))
    return meta


def _build_program(KBS, no_collectives=False):
    import concourse.bacc as bacc
    import concourse.mybir as mybir
    import concourse.tile as tile

    dt = mybir.dt
    nc = bacc.Bacc("TRN2", target_bir_lowering=False, debug=False,
                   num_devices=NCORE)

    NCH = [sum(kb) for kb in KBS]
    META = [_chunk_meta(kb) for kb in KBS]

    P = nc.declare_dram_parameter
    xAT = P("xAT", [FIN, TN], dt.float16, isOutput=False)
    xBT = P("xBT", [FIN, TN], dt.float16, isOutput=False)
    xLA = P("xLA", [FIN, DPC], dt.float16, isOutput=False)
    xLB = P("xLB", [FIN, DPC], dt.float16, isOutput=False)
    w1 = P("w1", [3, FIN, WCOL], dt.float16, isOutput=False)
    # l2 post-projection lhsT per (etype, half): rows (h,d') h-major, cols dd.
    # Folds W2/8 (h'=8h compensation) and the 1/8 head-mean.
    w2s = P("w2s", [3, 2, BLK, D], dt.float16, isOutput=False)
    # l2 folded attention vectors: cols 0:8 wl2[0], 8:16 wl2[2], 16:24 wl2[1],
    # 24:48 wr2[0..2]
    w2f = P("w2f", [D, 48], dt.float16, isOutput=False)
    bm = P("bm", [2, BLK, D], dt.float32, isOutput=False)
    bm2 = P("bm2", [D, 2], dt.float32, isOutput=False)
    ident = P("ident", [BLK, BLK], dt.float16, isOutput=False)
    u16 = [P(f"u16_{g}", [BLK, NCH[g] * 8], dt.int16, isOutput=False)
           for g in range(3)]
    pp = [P(f"pp_{g}", [BLK, NCH[g] * 2 * BLK], dt.float8e4, isOutput=False)
          for g in range(3)]
    oA = P("oA", [D, DPC], dt.float32, isOutput=True)
    oB = P("oB", [D, DPC], dt.float32, isOutput=True)

    tabs = [nc.dram_tensor(f"table{i}", [TN, TW], dt.float16)
            for i in range(3)]
    t2tabs = [nc.dram_tensor(f"t2_{i}", [TN, T2S], dt.float16)
              for i in range(2)]  # 0: src A (el2 e0, e2), 1: src B (el2 e1)
    hT_loc = [nc.dram_tensor(f"hT{s}_loc", [D, DPC], dt.float16)
              for s in range(2)]
    hT_full = [nc.dram_tensor(f"hT{s}_full", [NCORE * D, DPC], dt.float16,
                              addr_space="Shared") for s in range(2)]

    with tile.TileContext(nc) as tc:
        from contextlib import ExitStack
        with ExitStack() as es:
            cpool = es.enter_context(tc.tile_pool(name="consts", bufs=1))
            wpool = es.enter_context(tc.tile_pool(name="wts", bufs=1))
            xpool = es.enter_context(tc.tile_pool(name="xt", bufs=2))
            tspool = es.enter_context(tc.tile_pool(name="tsb", bufs=3))
            gpool = es.enter_context(tc.tile_pool(name="g", bufs=5))
            ppool = es.enter_context(tc.tile_pool(name="p01", bufs=4))
            upool = es.enter_context(tc.tile_pool(name="u16", bufs=2))
            lpool = es.enter_context(tc.tile_pool(name="l", bufs=3))
            erpool = es.enter_context(tc.tile_pool(name="er", bufs=1))
            mpool = es.enter_context(tc.tile_pool(name="misc", bufs=3))
            accpool = es.enter_context(tc.tile_pool(name="acc", bufs=1))
            hpool = es.enter_context(tc.tile_pool(name="h", bufs=1))
            ps_proj = es.enter_context(
                tc.tile_pool(name="ps_p", bufs=3, space="PSUM"))
            ps_agg = es.enter_context(
                tc.tile_pool(name="ps_a", bufs=2, space="PSUM"))
            # pse (conv-time) and ptr (hstage-time) share one 2-slot pool/tag
            ps_ere = es.enter_context(
                tc.tile_pool(name="ps_e", bufs=2, space="PSUM"))
            ps_erp = es.enter_context(
                tc.tile_pool(name="ps_r", bufs=1, space="PSUM"))
            ps_tr = ps_ere

            accA = accpool.tile([BLK, NBLK, HD], dt.float32, tag="accA")
            accB = accpool.tile([BLK, NBLK, HD], dt.float32, tag="accB")
            bm_sb = cpool.tile([BLK, 2, D], dt.float32, tag="bm")
            nc.sync.dma_start(bm_sb[:], bm[:, :, :].rearrange("b p d -> p b d"))
            bm2_sb = cpool.tile([D, 2], dt.float32, tag="bm2")
            nc.sync.dma_start(bm2_sb[:], bm2[:, :])
            id_sb = cpool.tile([BLK, BLK], dt.float16, tag="id")
            nc.sync.dma_start(id_sb[:], ident[:, :])
            wt1 = wpool.tile([BLK, 3, 2, WCOL], dt.float16, tag="w1")
            nc.sync.dma_start(
                wt1[:], w1[:, :, :].rearrange("e (a k) o -> k e a o", k=BLK))
            w2s_sb = wpool.tile([BLK, 3, 2, D], dt.float16, tag="w2s")
            nc.sync.dma_start(
                w2s_sb[:], w2s[:, :, :, :].rearrange("e f k o -> k e f o"))
            w2f_sb = wpool.tile([D, 48], dt.float16, tag="w2f")
            nc.sync.dma_start(w2f_sb[:], w2f[:, :])

            cvt = [0]  # alternates PSUM->SBUF converts between ACT and DVE

            def convert(dst, src):
                cvt[0] ^= 1
                if cvt[0]:
                    nc.scalar.copy(dst, src)
                else:
                    nc.vector.tensor_copy(dst, src)

            def er_l1(xl_dram, etypes):
                """er blocks for local dst nodes from layer-1 inputs."""
                xl = xpool.tile([BLK, 2, DPC], dt.float16, tag="lhs")
                nc.sync.dma_start(
                    xl[:], xl_dram[:, :].rearrange("(a k) n -> k a n", k=BLK))
                ers = []
                for e in etypes:
                    ps = ps_erp.tile([BLK, NBLK * H], dt.float32, tag="erp")
                    for b in range(NBLK):
                        for a in range(2):
                            nc.tensor.matmul(
                                ps[:, b * H:(b + 1) * H],
                                xl[:, a, b * BLK:(b + 1) * BLK],
                                wt1[:, e, a, HD + H:WCOL],
                                start=(a == 0), stop=(a == 1))
                    er_sb = erpool.tile([BLK, NBLK * H], dt.float16,
                                        tag=f"er{e}")
                    nc.scalar.copy(er_sb[:], ps[:])
                    ers.append(er_sb)
                return ers

            def er_l2(hT_sb, e):
                ps = ps_erp.tile([BLK, NBLK * H], dt.float32, tag="erp")
                for b in range(NBLK):
                    nc.tensor.matmul(
                        ps[:, b * H:(b + 1) * H],
                        hT_sb[:, b * BLK:(b + 1) * BLK],
                        w2f_sb[:, 24 + e * 8:32 + e * 8],
                        start=True, stop=True)
                er_sb = erpool.tile([BLK, NBLK * H], dt.float16, tag=f"er2{e}")
                nc.scalar.copy(er_sb[:], ps[:])
                return er_sb

            def proj_l1(srcT, specs):
                """One pass over a layer-1 input: project into 1-2 tables.

                specs: [(tab, etype)] sharing this source.
                """
                for c in range(NCORE):
                    lhs = xpool.tile([BLK, 2, DPC], dt.float16, tag="lhs")
                    nc.sync.dma_start(
                        lhs[:],
                        srcT[:, c * DPC:(c + 1) * DPC]
                        .rearrange("(a k) n -> k a n", k=BLK))
                    for tab, e in specs:
                        for half in range(NBLK // KST):
                            stage = tspool.tile([BLK, KST, HD + H],
                                                dt.float16, tag="stage")
                            for t in range(KST):
                                tt = half * KST + t
                                ps = ps_proj.tile([BLK, HD + H], dt.float32,
                                                  tag="pp")
                                for a in range(2):
                                    nc.tensor.matmul(
                                        ps[:],
                                        lhs[:, a, tt * BLK:(tt + 1) * BLK],
                                        wt1[:, e, a, :HD + H],
                                        start=(a == 0), stop=(a == 1))
                                convert(stage[:, t, :], ps[:])
                            r0 = (c * NBLK + half * KST) * BLK
                            nc.sync.dma_start(
                                tab[r0:r0 + KST * BLK, :HD + H]
                                .rearrange("(j p) w -> p j w", p=BLK),
                                stage[:])

            def build_t2(src_full, tab, wloff, nel):
                """l2 table: rows [h(32 f16) | el2 per etype (8 each)| junk].
                4 blocks per PSUM tile so converts are batched."""
                for c in range(NCORE):
                    hsl = xpool.tile([D, DPC], dt.float16, tag="lhs",
                                     padded_shape=[BLK, 2 * DPC])
                    nc.sync.dma_start(hsl[:], src_full[c * D:(c + 1) * D, :])
                    stage = tspool.tile([BLK, NBLK, T2S], dt.float16,
                                        tag="st2", bufs=1)
                    for q in range(NBLK // 4):
                        psT = ps_proj.tile([BLK, BLK], dt.float16, tag="pp")
                        pse_ = ps_proj.tile([BLK, 4, nel], dt.float32,
                                            tag="pp",
                                            padded_shape=[BLK, 4, HD // 4])
                        for k in range(4):
                            b = q * 4 + k
                            nc.tensor.transpose(
                                psT[:, k * D:(k + 1) * D],
                                hsl[:, b * BLK:(b + 1) * BLK], id_sb[:D, :D])
                            nc.tensor.matmul(
                                pse_[:, k, :], hsl[:, b * BLK:(b + 1) * BLK],
                                w2f_sb[:, wloff:wloff + nel],
                                start=True, stop=True)
                        convert(stage[:, q * 4:(q + 1) * 4, 0:D],
                                psT[:].rearrange("p (k d) -> p k d", d=D))
                        convert(stage[:, q * 4:(q + 1) * 4, D:D + nel],
                                pse_[:])
                    nc.sync.dma_start(
                        tab[c * DPC:(c + 1) * DPC, :]
                        .rearrange("(b p) w -> p b w", p=BLK),
                        stage[:])

            def tail(psb, b, acc, first):
                rec = mpool.tile([BLK, H], dt.float32, tag="rec")
                nc.vector.tensor_scalar(rec[:], psb[:, HD:HD + H], DEN_EPS,
                                        None, mybir.AluOpType.max)
                nc.vector.reciprocal(rec[:], rec[:])
                recb = rec[:].unsqueeze(1).broadcast_to([BLK, D, H])
                dst = acc[:, b, :].rearrange("p (d h) -> p d h", h=H)
                num = psb[:, :HD].rearrange("p (d h) -> p d h", h=H)
                if first:
                    nc.vector.tensor_tensor(dst, num, recb,
                                            mybir.AluOpType.mult)
                else:
                    tmp = mpool.tile([BLK, HD], dt.float32, tag="tmp")
                    nc.vector.tensor_tensor(
                        tmp[:].rearrange("p (d h) -> p d h", h=H), num, recb,
                        mybir.AluOpType.mult)
                    nc.vector.tensor_tensor(acc[:, b, :], acc[:, b, :], tmp[:],
                                            mybir.AluOpType.add)

            def hfold(acc, b0, nb, bmi, hT):
                """Fold hstage per 4-block group into the l1 conv tail:
                head-sum + bias + relu + transpose into hT."""
                hs = hpool.tile([BLK, 4, D], dt.float32, tag="hs")
                nc.vector.tensor_reduce(
                    hs[:, :nb],
                    acc[:, b0:b0 + nb, :].rearrange(
                        "p b (d h) -> p b d h", h=H),
                    mybir.AxisListType.X, mybir.AluOpType.add)
                nc.vector.tensor_tensor(
                    hs[:, :nb], hs[:, :nb],
                    bm_sb[:, bmi, :].unsqueeze(1).broadcast_to([BLK, nb, D]),
                    mybir.AluOpType.add)
                hb = hpool.tile([BLK, 4, D], dt.float16, tag="hb")
                nc.scalar.activation(hb[:, :nb], hs[:, :nb],
                                     mybir.ActivationFunctionType.Relu)
                psT4 = ps_proj.tile([D, 4 * BLK], dt.float16, tag="pp")
                for k in range(nb):
                    nc.tensor.transpose(
                        psT4[:, k * BLK:(k + 1) * BLK], hb[:, k, :], id_sb[:])
                convert(hT[:, b0 * BLK:(b0 + nb) * BLK], psT4[:, :nb * BLK])

            LAST_USB = [None]

            def conv(g, tab, er_sb, acc, first, finalize=None):
                nch = NCH[g]
                meta = META[g]
                nslice = (nch + SL - 1) // SL
                u_sb = upool.tile([BLK, NCH[g] * 8], dt.int16, tag="usb")
                nc.sync.dma_start(u_sb[:], u16[g][:, :])
                LAST_USB[0] = u_sb
                hT = None
                if finalize is not None:
                    bmi, hT_loc_dram = finalize
                    hT = hpool.tile([D, DPC], dt.float16, tag=f"hT{bmi}")
                psb = None
                for s in range(nslice):
                    c0 = s * SL
                    ns = min(SL, nch - c0)
                    G = gpool.tile([BLK, SL, TW], dt.float16, tag="G")
                    for j0 in range(0, ns, GSUB):
                        nj = min(GSUB, ns - j0)
                        nc.gpsimd.dma_gather(
                            G[:, j0:j0 + nj, :], tab[:, :],
                            u_sb[:, (c0 + j0) * 8:(c0 + j0 + nj) * 8],
                            num_idxs=nj * BLK, num_idxs_reg=nj * BLK,
                            elem_size=TW)
                    pt = ppool.tile([BLK, SL, 2, BLK], dt.float8e4, tag="p01")
                    nc.sync.dma_start(
                        pt[:, :ns, :, :],
                        pp[g][:, c0 * 2 * BLK:(c0 + ns) * 2 * BLK]
                        .rearrange("p (c t x) -> p c t x", t=2, x=BLK))
                    pse = ps_ere.tile([BLK, SL * H], dt.float32, tag="pse")
                    for j in range(ns):
                        b = meta[c0 + j][0]
                        nc.tensor.matmul(
                            pse[:, j * H:(j + 1) * H], pt[:, j, 1, :],
                            er_sb[:, b * H:(b + 1) * H], start=True, stop=True)
                    lt = lpool.tile([BLK, SL * H], dt.float16, tag="lt")
                    nc.vector.tensor_tensor(
                        lt[:, :ns * H].rearrange("p (c h) -> p c h", h=H),
                        G[:, :ns, ELOFF:ELOFF + H],
                        pse[:, :ns * H].rearrange("p (c h) -> p c h", h=H),
                        mybir.AluOpType.add)
                    l5 = lpool.tile([BLK, SL * H], dt.float16, tag="l5")
                    nc.vector.tensor_scalar(l5[:, :ns * H], lt[:, :ns * H],
                                            NEG, None, mybir.AluOpType.mult)
                    l2 = lpool.tile([BLK, SL * H], dt.float16, tag="l2")
                    nc.vector.tensor_tensor(l2[:, :ns * H], lt[:, :ns * H],
                                            l5[:, :ns * H],
                                            mybir.AluOpType.max)
                    nc.scalar.activation(
                        G[:, :ns, ELOFF:ELOFF + H],
                        l2[:, :ns * H].rearrange("p (c h) -> p c h", h=H),
                        mybir.ActivationFunctionType.Exp)
                    wbc = (G[:, :ns, ELOFF:ELOFF + H].unsqueeze(2)
                           .broadcast_to([BLK, ns, D, H]))
                    g4 = G[:, :ns, :HD].rearrange("p c (d h) -> p c d h", h=H)
                    nc.vector.tensor_tensor(g4, g4, wbc, mybir.AluOpType.mult)
                    for j in range(ns):
                        b, fst, lst = meta[c0 + j]
                        if fst:
                            psb = ps_agg.tile([BLK, HD + H], dt.float32,
                                              tag="psb")
                        nc.tensor.matmul(psb[:], pt[:, j, 0, :],
                                         G[:, j, :HD + H],
                                         start=fst, stop=lst)
                        if lst:
                            tail(psb, b, acc, first)
                            if finalize is not None and (
                                    b % 4 == 3 or b == NBLK - 1):
                                b0 = (b // 4) * 4
                                hfold(acc, b0, b - b0 + 1, finalize[0], hT)
                if finalize is not None:
                    nc.sync.dma_start(finalize[1][:, :], hT[:])
                return hT

            def tail2_mini(nsbA, b0, nb, accT, e, first):
                """l2 tail for blocks [b0, b0+nb): divide by den, transpose,
                project with head-mean folded into w2s."""
                rec = mpool.tile([BLK, 4, H], dt.float16, tag="recA")
                nc.vector.tensor_scalar(
                    rec[:, :nb], nsbA[:, b0:b0 + nb, HD:HD + H],
                    DEN_EPS, None, mybir.AluOpType.max)
                with nc.allow_low_precision(
                        reason="f16 softmax-denominator reciprocal; den is "
                               "O(1..100) so rel err ~5e-4 << 2e-2 gate"):
                    nc.vector.reciprocal(rec[:, :nb], rec[:, :nb])
                recx = mpool.tile([BLK, 4, H, 2], dt.float16, tag="recx")
                nc.vector.tensor_copy(
                    recx[:, :nb], rec[:, :nb].unsqueeze(3).broadcast_to(
                        [BLK, nb, H, 2]))
                nsd = nsbA[:, b0:b0 + nb, :HD].rearrange(
                    "p b (h a x) -> p b h a x", h=H, x=2)
                nc.vector.tensor_tensor(
                    nsd, nsd,
                    recx[:, :nb].unsqueeze(3).broadcast_to(
                        [BLK, nb, H, D // 2, 2]),
                    mybir.AluOpType.mult)
                for b in range(b0, b0 + nb):
                    psF = ps_erp.tile([D, BLK], dt.float32, tag="erp")
                    for half in range(2):
                        psT = ps_proj.tile([BLK, BLK], dt.float16, tag="pp")
                        nc.tensor.transpose(
                            psT[:],
                            nsbA[:, b, half * BLK:(half + 1) * BLK], id_sb[:])
                        sT = mpool.tile([BLK, BLK], dt.float16, tag="sT")
                        convert(sT[:], psT[:])
                        nc.tensor.matmul(psF[:], w2s_sb[:, e, half, :], sT[:],
                                         start=(half == 0), stop=(half == 1))
                    if first:
                        nc.scalar.copy(accT[:, b, :], psF[:])
                    else:
                        nc.vector.tensor_tensor(accT[:, b, :], accT[:, b, :],
                                                psF[:], mybir.AluOpType.add)

            def conv2h_dma(g, tab, u_sb, s):
                """Issue the gather + one-hot load for slice s; returns
                (G2, pt, ns) for the compute half."""
                nch = NCH[g]
                c0 = s * SL
                ns = min(SL, nch - c0)
                G2 = gpool.tile([BLK, SL, T2W], dt.float16, tag="G")
                for j0 in range(0, ns, GSUB):
                    nj = min(GSUB, ns - j0)
                    nc.gpsimd.dma_gather(
                        G2[:, j0:j0 + nj, :], tab[:, 0:T2W],
                        u_sb[:, (c0 + j0) * 8:(c0 + j0 + nj) * 8],
                        num_idxs=nj * BLK, num_idxs_reg=nj * BLK,
                        elem_size=T2W, elem_step=T2S)
                pt = ppool.tile([BLK, SL, 2, BLK], dt.float8e4, tag="p01")
                nc.sync.dma_start(
                    pt[:, :ns, :, :],
                    pp[g][:, c0 * 2 * BLK:(c0 + ns) * 2 * BLK]
                    .rearrange("p (c t x) -> p c t x", t=2, x=BLK))
                return (G2, pt, ns)

            def conv2h_prefetch(g, tab, u_sb, k):
                nslice = (NCH[g] + SL - 1) // SL
                return {s: conv2h_dma(g, tab, u_sb, s)
                        for s in range(min(k, nslice))}

            def conv2h(g, tab, er_sb, accT, e, eloff, first,
                       out_spec=None, u_sb_in=None, prefetched=None):
                """l2 conv: gather h-rows, aggregate w*h per head, project."""
                nch = NCH[g]
                meta = META[g]
                nslice = (nch + SL - 1) // SL
                if u_sb_in is None:
                    u_sb = upool.tile([BLK, NCH[g] * 8], dt.int16, tag="usb")
                    nc.sync.dma_start(u_sb[:], u16[g][:, :])
                else:
                    u_sb = u_sb_in
                nsbA = hpool.tile([BLK, NBLK, HD + H], dt.float16, tag="nsbA")
                psb = None
                for s in range(nslice):
                    c0 = s * SL
                    if prefetched is not None and s in prefetched:
                        G2, pt, ns = prefetched[s]
                    else:
                        G2, pt, ns = conv2h_dma(g, tab, u_sb, s)
                    pse = ps_ere.tile([BLK, SL * H], dt.float32, tag="pse")
                    for j in range(ns):
                        b = meta[c0 + j][0]
                        nc.tensor.matmul(
                            pse[:, j * H:(j + 1) * H], pt[:, j, 1, :],
                            er_sb[:, b * H:(b + 1) * H], start=True, stop=True)
                    lt = lpool.tile([BLK, SL * H], dt.float16, tag="lt")
                    nc.vector.tensor_tensor(
                        lt[:, :ns * H].rearrange("p (c h) -> p c h", h=H),
                        G2[:, :ns, eloff:eloff + H],
                        pse[:, :ns * H].rearrange("p (c h) -> p c h", h=H),
                        mybir.AluOpType.add)
                    l5 = lpool.tile([BLK, SL * H], dt.float16, tag="l5")
                    nc.vector.tensor_scalar(l5[:, :ns * H], lt[:, :ns * H],
                                            NEG, None, mybir.AluOpType.mult)
                    l2 = lpool.tile([BLK, SL * H], dt.float16, tag="l2")
                    nc.vector.tensor_tensor(l2[:, :ns * H], lt[:, :ns * H],
                                            l5[:, :ns * H],
                                            mybir.AluOpType.max)
                    Gw = gpool.tile([BLK, SL, HD + H], dt.float16, tag="G",
                                    padded_shape=[BLK, SL, TW])
                    nc.scalar.activation(
                        Gw[:, :ns, HD:HD + H],
                        l2[:, :ns * H].rearrange("p (c h) -> p c h", h=H),
                        mybir.ActivationFunctionType.Exp)
                    wx2 = lpool.tile([BLK, SL, H, 2], dt.float16, tag="wx2")
                    nc.vector.tensor_copy(
                        wx2[:, :ns],
                        Gw[:, :ns, HD:HD + H].unsqueeze(3)
                        .broadcast_to([BLK, ns, H, 2]))
                    nc.vector.tensor_tensor(
                        Gw[:, :ns, :HD]
                        .rearrange("p c (h a b) -> p c h a b", a=D // 2, b=2),
                        G2[:, :ns, 0:D]
                        .rearrange("p c (a b) -> p c a b", b=2)
                        .unsqueeze(2)
                        .broadcast_to([BLK, ns, H, D // 2, 2]),
                        wx2[:, :ns].unsqueeze(3)
                        .broadcast_to([BLK, ns, H, D // 2, 2]),
                        mybir.AluOpType.mult)
                    for j in range(ns):
                        b, fst, lst = meta[c0 + j]
                        if fst:
                            psb = ps_agg.tile([BLK, HD + H], dt.float32,
                                              tag="psb")
                        nc.tensor.matmul(psb[:], pt[:, j, 0, :],
                                         Gw[:, j, :HD + H],
                                         start=fst, stop=lst)
                        if lst:
                            convert(nsbA[:, b, :], psb[:])
                            if b % 4 == 3 or b == NBLK - 1:
                                b0 = (b // 4) * 4
                                nb = b - b0 + 1
                                tail2_mini(nsbA, b0, nb, accT, e, first)
                                if out_spec is not None:
                                    col, od = out_spec
                                    nc.vector.tensor_scalar(
                                        accT[:, b0:b0 + nb, :],
                                        accT[:, b0:b0 + nb, :],
                                        bm2_sb[:, col:col + 1],
                                        None, mybir.AluOpType.add)
                                    nc.sync.dma_start(
                                        od[:, b0 * BLK:(b0 + nb) * BLK],
                                        accT[:, b0:b0 + nb, :]
                                        .rearrange("p b x -> p (b x)"))

            def ostage2(accT, col, out_dram):
                nc.vector.tensor_scalar(accT[:], accT[:],
                                        bm2_sb[:, col:col + 1],
                                        None, mybir.AluOpType.add)
                nc.sync.dma_start(out_dram[:, :],
                                  accT[:].rearrange("p b x -> p (b x)"))

            def gather_h(s):
                if no_collectives:
                    # timing-model variant: replicate local h into all slots
                    for c in range(NCORE):
                        nc.sync.dma_start(
                            hT_full[s][c * D:(c + 1) * D, :], hT_loc[s][:, :])
                else:
                    nc.gpsimd.collective_compute(
                        "AllGather", mybir.AluOpType.bypass,
                        replica_groups=[list(range(NCORE))],
                        ins=[hT_loc[s][:, :]], outs=[hT_full[s][:, :]])

            # ---------------- layer 1 ----------------
            er1, er2 = er_l1(xLA, [1, 2])   # dst-A etypes
            (er0,) = er_l1(xLB, [0])        # dst-B etype
            proj_l1(xAT, [(tabs[0], 0), (tabs[2], 2)])
            proj_l1(xBT, [(tabs[1], 1)])
            hTb = conv(0, tabs[0], er0, accB, True, finalize=(1, hT_loc[1]))
            gather_h(1)
            er0b = er_l2(hTb, 0)   # dst B
            build_t2(hT_full[1], t2tabs[1], 16, 8)    # src hB (el2 e1)
            conv(2, tabs[2], er2, accA, True)
            hTa = conv(1, tabs[1], er1, accA, False,
                       finalize=(0, hT_loc[0]))
            gather_h(0)

            # ---------------- layer 2 ----------------
            er1b = er_l2(hTa, 1)   # dst A
            accTA = accpool.tile([D, NBLK, BLK], dt.float32, tag="accA")
            accTB = accpool.tile([D, NBLK, BLK], dt.float32, tag="accB")
            build_t2(hT_full[0], t2tabs[0], 0, 16)    # src hA (el2 e0, e2)
            er2b = er_l2(hTa, 2)   # dst A
            conv2h(1, t2tabs[1], er1b, accTA, 1, D, True,
                   u_sb_in=LAST_USB[0])
            conv2h(0, t2tabs[0], er0b, accTB, 0, D, True)
            conv2h(2, t2tabs[0], er2b, accTA, 2, D + H, False)
            ostage2(accTA, 0, oA)
            ostage2(accTB, 1, oB)

    nc.compile()
    return nc


_CACHE = {}


def _prep_inputs(inputs):
    f32 = np.float32
    xA = np.asarray(inputs["xA"], f32)
    xB = np.asarray(inputs["xB"], f32)
    W1 = np.asarray(inputs["W1"], f32)
    al1 = np.asarray(inputs["al1"], f32)
    ar1 = np.asarray(inputs["ar1"], f32)
    b1 = np.asarray(inputs["b1"], f32)
    W2 = np.asarray(inputs["W2"], f32)
    al2 = np.asarray(inputs["al2"], f32)
    ar2 = np.asarray(inputs["ar2"], f32)
    b2 = np.asarray(inputs["b2"], f32)
    uv = [(np.asarray(inputs["u0"]), np.asarray(inputs["v0"])),
          (np.asarray(inputs["u1"]), np.asarray(inputs["v1"])),
          (np.asarray(inputs["u2"]), np.asarray(inputs["v2"]))]

    graphs = [_prep_graph(u, v) for u, v in uv]
    KBS = tuple(g[0] for g in graphs)

    def bf(x):
        return np.ascontiguousarray(x.astype(F16))

    xATn = np.zeros((FIN, TN), f32)
    xATn[:, :N] = xA.T
    xBTn = np.zeros((FIN, TN), f32)
    xBTn[:, :N] = xB.T
    # packed weights: [W (d,h) | W.al | W.ar]
    w1n = np.stack([np.concatenate(
        [W1[e][:, _PERM], _fold(W1[e], al1[e]), _fold(W1[e], ar1[e])], 1)
        for e in range(3)])
    # l2 post-projection lhsT: rows (h, d') h-major, cols dd; W2/64 folds the
    # 8x-h compensation (1/8) and the head-mean (1/8).
    w2sn = np.zeros((3, 2, BLK, D), f32)
    for e in range(3):
        resh = (W2[e] / 64.0).reshape(D, H, D)      # [d', h, dd]
        for half in range(2):
            w2sn[e, half] = resh[:, half * 4:(half + 1) * 4, :] \
                .transpose(1, 0, 2).reshape(BLK, D)
    wl2 = [_fold(W2[e], al2[e]) / 8.0 for e in range(3)]
    wr2 = [_fold(W2[e], ar2[e]) / 8.0 for e in range(3)]
    w2fn = np.concatenate(
        [wl2[0], wl2[2], wl2[1], wr2[0], wr2[1], wr2[2]], 1)
    bmn = np.zeros((2, BLK, D), f32)
    bmn[0] = 8.0 * (b1[1] + b1[2]).reshape(H, D).mean(0)
    bmn[1] = 8.0 * b1[0].reshape(H, D).mean(0)
    bm2n = np.zeros((D, 2), f32)
    bm2n[:, 0] = (b2[1] + b2[2]).reshape(H, D).mean(0)
    bm2n[:, 1] = b2[0].reshape(H, D).mean(0)

    shared = dict(xAT=bf(xATn), xBT=bf(xBTn), w1=bf(w1n), w2s=bf(w2sn),
                  w2f=bf(w2fn), bm=bmn, bm2=bm2n,
                  ident=np.eye(BLK).astype(F16))

    in_maps = []
    for c in range(NCORE):
        m = dict(shared)
        xla = np.zeros((FIN, DPC), f32)
        xlb = np.zeros((FIN, DPC), f32)
        lo = c * DPC
        hi = min(N, lo + DPC)
        if hi > lo:
            xla[:, :hi - lo] = xA.T[:, lo:hi]
            xlb[:, :hi - lo] = xB.T[:, lo:hi]
        m["xLA"] = bf(xla)
        m["xLB"] = bf(xlb)
        for g in range(3):
            cd = graphs[g][1][c]
            m[f"u16_{g}"] = cd["u16"]
            m[f"pp_{g}"] = cd["pp"]
        in_maps.append(m)
    return KBS, in_maps


LAST_EXEC_NS = None


def kernel(**inputs):
    import os
    import time
    from concourse.bass_utils import run_bass_kernel_spmd

    global LAST_EXEC_NS
    KBS, in_maps = _prep_inputs(inputs)
    if KBS not in _CACHE:
        # the Tile scheduler is occasionally order-sensitive; retry the build
        last = None
        for _ in range(8):
            try:
                _CACHE[KBS] = _build_program(KBS)
                break
            except Exception as e:  # noqa: BLE001
                last = e
        else:
            raise last
    nc = _CACHE[KBS]

    res = run_bass_kernel_spmd(nc, in_maps, list(range(NCORE)))
    if os.environ.get("BASS_BENCH"):
        best = None
        for _ in range(int(os.environ.get("BASS_BENCH_ITERS", "3"))):
            t0 = time.perf_counter()
            run_bass_kernel_spmd(nc, in_maps, list(range(NCORE)))
            dt_s = time.perf_counter() - t0
            best = dt_s if best is None else min(best, dt_s)
        LAST_EXEC_NS = int(best * 1e9)
        try:
            r2 = run_bass_kernel_spmd(nc, in_maps, list(range(NCORE)),
                                      trace=True)
            if r2.exec_time_ns:
                LAST_EXEC_NS = int(r2.exec_time_ns)
        except Exception as e:
            print("trace unavailable:", type(e).__name__, str(e)[:120])
    f32 = np.float32
    outA = np.zeros((N, D), f32)
    outB = np.zeros((N, D), f32)
    for c in range(NCORE):
        lo = c * DPC
        hi = min(N, lo + DPC)
        outA[lo:hi] = np.asarray(res.results[c]["oA"], f32) \
            .reshape(D, DPC).T[:hi - lo]
        outB[lo:hi] = np.asarray(res.results[c]["oB"], f32) \
            .reshape(D, DPC).T[:hi - lo]
    return np.stack([outA, outB]).astype(np.float32)

